# revision 21
# baseline (speedup 1.0000x reference)
"""BatchRenorm2d forward on 8 TRN2 NeuronCores.

Full input [16, 64, 256, 256] f32. Data-parallel over batch: core i takes
batches [2i, 2i+1], viewed as [128, 65536] (partition = b_local*64 + c).

Per core:
  pass 1   stream 32 column tiles [128, 2048]; DVE reduce_sum and ACT
           Square-with-accumulate give per-partition sum / sumsq. The last
           RES tiles stay resident in SBUF.
  stats    fold the two local batches (partition p and p+64 hold the same
           channel), AllReduce a [64, 2] (sum, sumsq) buffer across the 8
           cores, then derive scale = 1/sqrt(var+eps), bias = -mu*scale.
  pass 2   normalize resident tiles in place, re-stream the non-resident
           tiles. Loads issue on the Sync HWDGE ring, stores on the Scalar
           ring so blocked stores never head-of-line-block prefetch.
"""

import numpy as np
import concourse.bass as bass
import concourse.bacc as bacc
import concourse.tile as tile
import concourse.mybir as mybir
from concourse import bass_utils

N_CORES = 8
B, C, H, W = 16, 64, 256, 256
PB = B // N_CORES          # batches per core
P = PB * C                 # 128 SBUF partitions
F = H * W                  # 65536 elements per (b, c) row
N_TOT = B * H * W          # reduction count per channel
EPS = 1e-5
T = 2048                   # tile free-dim size
NT = F // T                # 32 tiles per pass
RES = 20                   # tiles kept resident between the passes
STREAM_BUFS = 5

FP = mybir.dt.float32
AX = mybir.AxisListType
ALU = mybir.AluOpType
ACT = mybir.ActivationFunctionType

_nc_cache = None


def _build():
    nc = bacc.Bacc("TRN2", target_bir_lowering=False, debug=False,
                   num_devices=N_CORES)
    x = nc.dram_tensor("x", [P, F], FP, kind="ExternalInput").ap()
    y = nc.dram_tensor("y", [P, F], FP, kind="ExternalOutput").ap()

    n_stream = NT - RES

    with tile.TileContext(nc) as tc:
        with tc.tile_pool(name="stream", bufs=STREAM_BUFS) as stream, \
             tc.tile_pool(name="resp", bufs=RES) as resp, \
             tc.tile_pool(name="scratchp", bufs=1, space="PSUM") as scratchp, \
             tc.tile_pool(name="statsp", bufs=1) as statsp, \
             tc.tile_pool(name="dram", bufs=1, space="DRAM") as dram:

            sums = statsp.tile([P, NT], FP)
            sqs = statsp.tile([P, NT], FP)
            scratch = scratchp.tile([P, T], FP)

            # Pass 1: per-partition sum (DVE) and sum-of-squares (ACT).
            resident = {}
            for j in range(NT):
                if j >= n_stream:
                    t = resp.tile([P, T], FP, name=f"r{j}", tag="res")
                    resident[j] = t
                else:
                    t = stream.tile([P, T], FP, name=f"t{j}", tag="stream")
                nc.sync.dma_start(t[:], x[:, j * T:(j + 1) * T])
                nc.vector.reduce_sum(sums[:, j:j + 1], t[:], axis=AX.X)
                nc.scalar.activation(scratch[:], t[:], ACT.Square,
                                     accum_out=sqs[:, j:j + 1])

            sq = statsp.tile([P, 2], FP)
            nc.vector.reduce_sum(sq[:, 0:1], sums[:], axis=AX.X)
            nc.vector.reduce_sum(sq[:, 1:2], sqs[:], axis=AX.X)

            # Fold the two local batches: channel c lives on partitions c
            # and c + 64. Stats-path DMAs ride the scalar HWDGE ring: they
            # never block anything that does not already need the stats,
            # and HWDGE dispatch is much faster than gpsimd SWDGE.
            tmp = statsp.tile([64, 2], FP)
            nc.scalar.dma_start(tmp[:], sq[64:128, :])
            part = statsp.tile([64, 2], FP)
            nc.vector.tensor_add(part[:], sq[0:64, :], tmp[:])
            # Pre-scale by 1/N so the AllReduce directly yields (mu, E[x^2]).
            nc.vector.tensor_scalar_mul(part[:], part[:], 1.0 / N_TOT)

            # AllReduce per-channel (mean, mean-square) across the 8 cores.
            cc_in = dram.tile([64, 2], FP)
            cc_out = dram.tile([64, 2], FP, addr_space="Shared")
            nc.scalar.dma_start(cc_in[:], part[:])
            nc.gpsimd.collective_compute(
                "AllReduce", ALU.add,
                replica_groups=[list(range(N_CORES))],
                ins=[cc_in.opt()], outs=[cc_out.opt()],
            )
            tot = statsp.tile([P, 2], FP)
            nc.scalar.dma_start(tot[0:64, :], cc_out[:])
            nc.scalar.dma_start(tot[64:128, :], cc_out[:])

            # scale = 1/sqrt(var + eps), bias = -mu * scale, per partition.
            # Minimize ACT<->DVE crossings: DVE -> ACT(sqrt) -> DVE.
            mu = tot[:, 0:1]
            musq = statsp.tile([P, 1], FP)
            var = statsp.tile([P, 1], FP)
            std = statsp.tile([P, 1], FP)
            inv = statsp.tile([P, 1], FP)
            negmu = statsp.tile([P, 1], FP)
            biasv = statsp.tile([P, 1], FP)
            epst = statsp.tile([P, 1], FP)
            nc.vector.memset(epst[:], EPS)
            nc.vector.tensor_mul(musq[:], mu, mu)
            nc.vector.tensor_sub(var[:], tot[:, 1:2], musq[:])
            nc.vector.tensor_scalar_mul(negmu[:], mu, -1.0)
            nc.scalar.activation(std[:], var[:], ACT.Sqrt, bias=epst[:])
            nc.vector.reciprocal(inv[:], std[:])
            nc.vector.tensor_mul(biasv[:], negmu[:], inv[:])

            def normalize(t, j, k, chunks=1):
                cw = T // chunks
                for c in range(chunks):
                    lo, hi = c * cw, (c + 1) * cw
                    if k % 2 == 1:
                        nc.scalar.activation(t[:, lo:hi], t[:, lo:hi],
                                             ACT.Identity,
                                             bias=biasv[:], scale=inv[:])
                        nc.scalar.dma_start(y[:, j * T + lo:j * T + hi],
                                            t[:, lo:hi])
                    else:
                        nc.vector.tensor_scalar(t[:, lo:hi], t[:, lo:hi],
                                                negmu[:], inv[:],
                                                op0=ALU.add, op1=ALU.mult)
                        nc.scalar.dma_start(y[:, j * T + lo:j * T + hi],
                                            t[:, lo:hi])

            # Pass 2: interleave resident tiles (no reload, drain right
            # after the collective) with re-streamed tiles so stream slots
            # free steadily. Loads on sync, stores on scalar.
            order = []
            ri, si = n_stream, 0
            for k in range(NT):
                if (k % 2 == 0 and ri < NT) or si >= n_stream:
                    order.append(ri)
                    ri += 1
                else:
                    order.append(si)
                    si += 1
            n_reload = 0
            for k, j in enumerate(order):
                if j >= n_stream:
                    normalize(resident[j], j, k)
                else:
                    # First STREAM_BUFS reloads prefetch during the
                    # collective from the stream pool; later ones reuse
                    # resident-pool slots freed by already-stored tiles,
                    # deepening the steady-state pipeline at no SBUF cost.
                    if n_reload < STREAM_BUFS:
                        t = stream.tile([P, T], FP, name=f"o{j}",
                                        tag="stream")
                    else:
                        t = resp.tile([P, T], FP, name=f"o{j}", tag="res")
                    n_reload += 1
                    nc.sync.dma_start(t[:], x[:, j * T:(j + 1) * T])
                    normalize(t, j, k)

    nc.compile()
    return nc


def _get_nc():
    global _nc_cache
    if _nc_cache is None:
        _nc_cache = _build()
    return _nc_cache


def _run(inputs, trace=False, **kwargs):
    nc = _get_nc()
    x = np.ascontiguousarray(np.asarray(inputs, dtype=np.float32))
    shards = x.reshape(N_CORES, P, F)
    in_maps = [{"x": shards[i]} for i in range(N_CORES)]
    res = bass_utils.run_bass_kernel_spmd(
        nc, in_maps, core_ids=list(range(N_CORES)), trace=trace, **kwargs)
    out = np.stack([res.results[i]["y"] for i in range(N_CORES)], axis=0)
    return out.reshape(B, C, H, W), res


def kernel(inputs):
    out, _ = _run(inputs)
    return out
